# revision 1
# baseline (speedup 1.0000x reference)
"""Multi-head attention (B=4, S=2048, D=1024, H=16) on 8 Trainium2 NeuronCores.

Sharding: core i handles batch b = i // 2, head-group g = i % 2 (8 heads,
model dims [512g, 512g+512)).  Wq/Wk/Wv are split column-wise by head group,
Wo row-wise; each core computes a partial output out_partial.T [1024, 2048]
and the host sums the two partials per batch (the "all-reduce" of the
row-parallel out projection), adds bo, and transposes.

Device dataflow (everything stays transposed; no on-device transposes):
  YqT/YkT [o_local, s] = (WT)^T @ XT          (per-head-dim on partitions)
  Yv      [s, o_local] with a ones column per head
  logitsT [s_k, s_q]   = khT^T @ qhT          (K=64, head pairs packed into
                                               PE rows 0-63 / 64-127)
  el      = exp(logitsT)  (no max subtraction; masked entries get -1e9 and
                           underflow to exactly 0)
  av      [65, s_q]    = [vh | 1]^T @ el      (row 64 = sum of exp)
  yot     = av[0:64] * broadcast(1 / av[64])
  outT    [m, s]      += WoT^T @ yot          (partial; summed on host)

All matmuls run in float32r (single-pass fp32, ~1.3e-4 rel err, 4x faster
than fp32 on the PE).
"""

import os
import sys
import time
from contextlib import ExitStack

import numpy as np

for _p in ("/opt/trn_rl_repo", "/root/.axon_site/_ro/trn_rl_repo"):
    if os.path.isdir(_p) and _p not in sys.path:
        sys.path.insert(0, _p)
        break

import concourse.bass as bass  # noqa: E402
import concourse.mybir as mybir  # noqa: E402
import concourse.tile as tile  # noqa: E402
from concourse import bacc, bass_utils  # noqa: E402
from concourse.bass import ts  # noqa: E402

B, S, D = 4, 2048, 1024
H, DH = 16, 64
NCORES = 8
GROUPS = 2
O = D // GROUPS          # 512 local head dims per core
HL = H // GROUPS         # 8 local heads
P = 128
SQ = 512                 # s_q block size
NB = S // SQ             # 4 blocks
NKC = S // P             # 16 s_k chunks
KO = D // P              # 8 contraction k-tiles for qkv projections
F32 = mybir.dt.float32
F32R = mybir.dt.float32r
BF16 = mybir.dt.bfloat16
EXP = mybir.ActivationFunctionType.Exp
ADD = mybir.AluOpType.add
MULT = mybir.AluOpType.mult

# matmul operand dtype: "f32r" (default, ~1.3e-4 rel err) or "bf16" (faster
# LDWEIGHTS, ~5e-3 rel err)
DT_MODE = os.environ.get("MHA_DTYPE", "f32r")
DT = BF16 if DT_MODE == "bf16" else F32R
# timing-only ablations: "", "noatt", "logitsonly", "noavdep", "nopreload"
ABLATE = os.environ.get("MHA_ABLATE", "")

LAST_RESULTS = None      # BassKernelResults of the last kernel() call
_BUILD_CACHE = {}


def _classify_mask(mask2d):
    """Per (s_q block, s_k chunk) tile classification from the actual mask.

    Returns (plan, mtiles): plan = (blocks, n_slots) where blocks[b] is a
    tuple of (chunk, slot) pairs to compute (slot None => no mask add), and
    mtiles [n, 128, SQ] are deduplicated transposed mask tiles pre-multiplied
    by -1e9.
    """
    blocks = []
    slot_of = {}
    slots = []
    for b in range(NB):
        lst = []
        for c in range(NKC):
            sub = mask2d[b * SQ:(b + 1) * SQ, c * P:(c + 1) * P]  # [s_q, s_k]
            if not sub.any():
                lst.append((c, None))
            elif (sub == 1.0).all():
                continue  # fully masked tile: exp underflows to 0, skip work
            else:
                t = np.ascontiguousarray(sub.T.astype(np.float32) * np.float32(-1e9))
                key = t.tobytes()
                if key not in slot_of:
                    slot_of[key] = len(slots)
                    slots.append(t)
                lst.append((c, slot_of[key]))
        assert lst, f"s_q block {b} fully masked; unsupported"
        blocks.append(tuple(lst))
    if slots:
        mtiles = np.stack(slots)
    else:
        mtiles = np.zeros((1, P, SQ), np.float32)
    return (tuple(blocks), len(slots)), mtiles


def _build(plan, reps=1):
    blocks, n_slots = plan
    nc = bacc.Bacc("TRN2", target_bir_lowering=False, debug=False,
                   num_devices=NCORES)

    xq = nc.dram_tensor("xq", [D, S], DT, kind="ExternalInput").ap()
    xk = nc.dram_tensor("xk", [D, S], DT, kind="ExternalInput").ap()
    xv = nc.dram_tensor("xv", [D, S], DT, kind="ExternalInput").ap()
    wq = nc.dram_tensor("wq", [D, O], DT, kind="ExternalInput").ap()
    wk = nc.dram_tensor("wk", [D, O], DT, kind="ExternalInput").ap()
    wv = nc.dram_tensor("wv", [D, O], DT, kind="ExternalInput").ap()
    wo = nc.dram_tensor("wo", [O, D], DT, kind="ExternalInput").ap()
    bqd = nc.dram_tensor("bq", [P, O // P], F32, kind="ExternalInput").ap()
    bkd = nc.dram_tensor("bk", [P, O // P], F32, kind="ExternalInput").ap()
    bvd = nc.dram_tensor("bv", [P, O], F32, kind="ExternalInput").ap()
    mtd = nc.dram_tensor("mtiles", [max(n_slots, 1), P, SQ], DT,
                         kind="ExternalInput").ap()
    ones_d = nc.dram_tensor("ones", [P, HL], DT, kind="ExternalInput").ap()
    ident_d = nc.dram_tensor("ident", [P, P], DT, kind="ExternalInput").ap()
    out = nc.dram_tensor("out", [D, S], F32, kind="ExternalOutput").ap()

    xq_r = xq.rearrange("(ko p) s -> p ko s", p=P)
    xk_r = xk.rearrange("(ko p) s -> p ko s", p=P)
    xv_r = xv.rearrange("(ko p) s -> p ko s", p=P)

    with tile.TileContext(nc) as tc, ExitStack() as ctx:
        if reps > 1:
            ctx.enter_context(tc.For_i(0, reps, 1))
        # ---- persistent pools ----
        ykp = ctx.enter_context(tc.tile_pool(name="yk", bufs=1))
        yvp = ctx.enter_context(tc.tile_pool(name="yv", bufs=1))
        cons = ctx.enter_context(tc.tile_pool(name="cons", bufs=1))
        wqp = ctx.enter_context(tc.tile_pool(name="wqp", bufs=1))
        xqp = ctx.enter_context(tc.tile_pool(name="xq", bufs=1))
        yqpool = ctx.enter_context(tc.tile_pool(name="yq", bufs=2))
        elpool = ctx.enter_context(tc.tile_pool(name="el", bufs=3))
        nrmpool = ctx.enter_context(tc.tile_pool(name="nrm", bufs=2))
        bcpool = ctx.enter_context(tc.tile_pool(name="bcp", bufs=2))
        psum = ctx.enter_context(tc.tile_pool(name="ps", bufs=2, space="PSUM"))

        ykt_s = [ykp.tile([P, O // P, SQ], DT, tag=f"ykt{i}", name=f"ykt{i}")
                 for i in range(S // SQ)]
        yv_tiles = [yvp.tile([P, HL, DH + 1], DT, tag=f"yv{i}", name=f"yv{i}")
                    for i in range(NKC)]

        # constants go on the gpsimd DMA queue so they don't serialize the
        # critical wk/xk/wq loads on the sync queue
        bq_sb = cons.tile([P, O // P], F32, tag="bq")
        nc.gpsimd.dma_start(bq_sb[:], bqd)
        bk_sb = cons.tile([P, O // P], F32, tag="bk")
        nc.gpsimd.dma_start(bk_sb[:], bkd)
        bv_sb = cons.tile([P, O], F32, tag="bv")
        nc.gpsimd.dma_start(bv_sb[:], bvd)
        ident_sb = cons.tile([P, P], DT, tag="ident")
        nc.gpsimd.dma_start(ident_sb[:], ident_d)
        mask_sb = []
        for i in range(n_slots):
            t = cons.tile([P, SQ], DT, tag=f"mask{i}", name=f"mask{i}")
            nc.gpsimd.dma_start(t[:], mtd[i])
            mask_sb.append(t)
        if ABLATE == "noavdep":
            elc = cons.tile([P, 2 * SQ], DT, tag="elc")
            nc.gpsimd.dma_start(elc[:, 0:SQ], mtd[0])
            nc.gpsimd.dma_start(elc[:, SQ:2 * SQ], mtd[0])
        wq_sb = wqp.tile([P, KO, O], DT, tag="wq")

        def qproj(b):
            xq_blk = xqp.tile([P, KO, SQ], DT, tag="xq")
            nc.gpsimd.dma_start(xq_blk[:], xq_r[:, :, ts(b, SQ)])
            yqt = yqpool.tile([P, O // P, SQ], DT, tag="yq")
            for oc in range(O // P):
                ps = psum.tile([P, SQ], F32, tag="qp")
                for ko in range(KO):
                    nc.tensor.matmul(ps[:], wq_sb[:, ko, ts(oc, P)],
                                     xq_blk[:, ko, :],
                                     start=(ko == 0), stop=(ko == KO - 1))
                nc.vector.tensor_scalar_add(yqt[:, oc, :], ps[:],
                                            bq_sb[:, oc:oc + 1])
            return yqt

        # ---- phase A: K-proj(sc0), Q-proj(0), V-proj, K-proj(sc1..3) ----
        with tc.tile_pool(name="wkv", bufs=1) as wpool, \
             tc.tile_pool(name="xin", bufs=2) as xpool:
            wk_sb = wpool.tile([P, KO, O], DT, tag="wk")
            nc.sync.dma_start(wk_sb[:], wk.rearrange("(ko p) o -> p ko o", p=P))
            wv_sb = wpool.tile([P, KO, O], DT, tag="wv")
            nc.gpsimd.dma_start(wv_sb[:], wv.rearrange("(ko p) o -> p ko o", p=P))

            def kproj(sc):
                xk_blk = xpool.tile([P, KO, SQ], DT, tag="xk")
                nc.sync.dma_start(xk_blk[:], xk_r[:, :, ts(sc, SQ)])
                for oc in range(O // P):
                    ps = psum.tile([P, SQ], F32, tag="qp")
                    for ko in range(KO):
                        nc.tensor.matmul(ps[:], wk_sb[:, ko, ts(oc, P)],
                                         xk_blk[:, ko, :],
                                         start=(ko == 0), stop=(ko == KO - 1))
                    nc.vector.tensor_scalar_add(ykt_s[sc][:, oc, :], ps[:],
                                                bk_sb[:, oc:oc + 1])

            def vproj4(g):  # V-proj for s chunks 4g..4g+3 from one DMA
                xv_blk = xpool.tile([P, KO, SQ], DT, tag="xk", name=f"xv{g}")
                nc.sync.dma_start(xv_blk[:], xv_r[:, :, ts(g, SQ)])
                for sub in range(SQ // P):
                    sc = 4 * g + sub
                    ps = psum.tile([P, O], F32, tag="qp")
                    for ko in range(KO):
                        nc.tensor.matmul(ps[:], xv_blk[:, ko, ts(sub, P)],
                                         wv_sb[:, ko, :],
                                         start=(ko == 0), stop=(ko == KO - 1))
                    yvt = yv_tiles[sc]
                    nc.vector.tensor_tensor(
                        yvt[:, :, 0:DH],
                        ps[:].rearrange("p (h d) -> p h d", d=DH),
                        bv_sb[:].rearrange("p (h d) -> p h d", d=DH),
                        ADD,
                    )
                    nc.gpsimd.dma_start(yvt[:, :, DH], ones_d)

            kproj(0)
            nc.sync.dma_start(wq_sb[:], wq.rearrange("(ko p) o -> p ko o", p=P))
            yqt = qproj(0)
            vproj4(0)
            for sc in range(1, S // SQ):
                kproj(sc)
                vproj4(sc)

        # ---- phase B: per-block attention + next Q-proj + out-proj ----
        with tc.tile_pool(name="yo", bufs=2) as yopool, \
             tc.tile_pool(name="wop", bufs=1) as wopool, \
             tc.tile_pool(name="ost", bufs=2) as ostpool:
            wo_sb = wopool.tile([P, O // P, D], DT, tag="wo")
            nc.sync.dma_start(wo_sb[:], wo.rearrange("(kc p) m -> p kc m", p=P))
            for b in range(NB):
                yot = yopool.tile([P, O // P, SQ], DT, tag="yo")
                chunks = blocks[b]
                first_c = chunks[0][0]
                last_c = chunks[-1][0]
                for t in range(O // P) if ABLATE != "noatt" else []:
                    av = [psum.tile([P, SQ], F32, tag="av", name=f"av{hh}")
                          for hh in range(2)]
                    for (c, slot) in chunks:
                        lp = psum.tile([P, 2 * SQ], F32, tag="lp")
                        for hh in range(2):
                            if slot is not None and ABLATE != "nopreload":
                                nc.tensor.matmul(
                                    lp[:, ts(hh, SQ)], ident_sb[:],
                                    mask_sb[slot][:], start=True, stop=False)
                            nc.tensor.matmul(
                                lp[:, ts(hh, SQ)],
                                ykt_s[c // 4][ts(hh, DH), t, ts(c % 4, P)],
                                yqt[ts(hh, DH), t, :],
                                start=(slot is None or ABLATE == "nopreload"),
                                stop=True,
                            )
                        if slot is not None and ABLATE == "nopreload":
                            for hh in range(2):
                                nc.vector.tensor_tensor(
                                    lp[:, ts(hh, SQ)], lp[:, ts(hh, SQ)],
                                    mask_sb[slot][:], ADD)
                        el = elpool.tile([P, 2 * SQ], DT, tag="el")
                        nc.scalar.activation(el[:], lp[:], EXP)
                        if ABLATE == "logitsonly":
                            continue
                        av_rhs = el if ABLATE != "noavdep" else elc
                        for hh in range(2):
                            nc.tensor.matmul(
                                av[hh][0:DH + 1, :],
                                yv_tiles[c][:, 2 * t + hh, :],
                                av_rhs[:, ts(hh, SQ)],
                                start=(c == first_c), stop=(c == last_c),
                            )
                    if ABLATE in ("logitsonly",):
                        continue
                    for hh in range(2):
                        rec = nrmpool.tile([1, SQ], F32, tag="rec")
                        nc.vector.reciprocal(rec[:], av[hh][DH:DH + 1, :])
                        bc = bcpool.tile([DH, SQ], F32, tag="bc")
                        nc.gpsimd.partition_broadcast(bc[:], rec[:])
                        nc.vector.tensor_tensor(
                            yot[ts(hh, DH), t, :], av[hh][0:DH, :], bc[:], MULT)

                if b + 1 < NB:
                    yqt = qproj(b + 1)

                # out-proj for this block: out[m, s] partial
                for mc in range(D // P):
                    ps = psum.tile([P, SQ], F32, tag="qp")
                    for kc in range(O // P):
                        nc.tensor.matmul(ps[:], wo_sb[:, kc, ts(mc, P)],
                                         yot[:, kc, :],
                                         start=(kc == 0), stop=(kc == O // P - 1))
                    ot = ostpool.tile([P, SQ], F32, tag="ot")
                    nc.vector.tensor_copy(ot[:], ps[:])
                    nc.sync.dma_start(out[ts(mc, P), ts(b, SQ)], ot[:])

    nc.compile()
    return nc


def _get_nc(plan):
    if plan not in _BUILD_CACHE:
        _BUILD_CACHE[plan] = _build(plan)
    return _BUILD_CACHE[plan]


def _host_prep(q, k, v, mask, Wq, bq, Wk, bk, Wv, bv, Wo, bo):
    q = np.asarray(q, np.float32)
    k = np.asarray(k, np.float32)
    v = np.asarray(v, np.float32)
    mask2d = np.asarray(mask, np.float32).reshape(S, S)
    Wq = np.asarray(Wq, np.float32)
    Wk = np.asarray(Wk, np.float32)
    Wv = np.asarray(Wv, np.float32)
    Wo = np.asarray(Wo, np.float32)
    bq = np.asarray(bq, np.float32)
    bk = np.asarray(bk, np.float32)
    bv = np.asarray(bv, np.float32)

    plan, mtiles = _classify_mask(mask2d)

    if DT == BF16:
        import ml_dtypes
        cst = lambda a: np.ascontiguousarray(a, dtype=ml_dtypes.bfloat16)
    else:
        cst = lambda a: np.ascontiguousarray(a, dtype=np.float32)

    in_maps = []
    for core in range(NCORES):
        b, g = divmod(core, GROUPS)
        sl = slice(g * O, (g + 1) * O)
        in_maps.append({
            "xq": cst(q[b].T),
            "xk": cst(k[b].T),
            "xv": cst(v[b].T),
            "wq": cst((Wq[sl, :] * 0.125).T),
            "wk": cst(Wk[sl, :].T),
            "wv": cst(Wv[sl, :].T),
            "wo": cst(Wo[:, sl].T),
            "bq": np.ascontiguousarray((bq[sl] * 0.125).reshape(O // P, P).T),
            "bk": np.ascontiguousarray(bk[sl].reshape(O // P, P).T),
            "bv": np.ascontiguousarray(np.tile(bv[sl].reshape(1, O), (P, 1))),
            "mtiles": cst(mtiles),
            "ones": cst(np.ones((P, HL), np.float32)),
            "ident": cst(np.eye(P, dtype=np.float32)),
        })
    return plan, in_maps


def kernel(q, k, v, mask, Wq, bq, Wk, bk, Wv, bv, Wo, bo):
    global LAST_RESULTS
    plan, in_maps = _host_prep(q, k, v, mask, Wq, bq, Wk, bk, Wv, bv, Wo, bo)
    nc = _get_nc(plan)
    bo = np.asarray(bo, np.float32)

    t0 = time.time()
    res = bass_utils.run_bass_kernel_spmd(nc, in_maps,
                                          core_ids=list(range(NCORES)))
    LAST_RESULTS = res
    LAST_RESULTS.wall_s = time.time() - t0

    outs = []
    bo64 = bo.astype(np.float64)
    for b in range(B):
        s = (res.results[GROUPS * b]["out"].astype(np.float64)
             + res.results[GROUPS * b + 1]["out"].astype(np.float64))
        outs.append(s.T + bo64)
    return np.stack(outs).astype(np.float32)

